# revision 32
# baseline (speedup 1.0000x reference)
"""Trainium2 Bass kernel for the ragged-sequence cross-attention module.

Math (reference):
    f       = Wf @ f_pre_in.T + bf                      (H, M)
    b_feat  = Wb @ b_pre_in[g] + bb                     per graph (H, N)
    bv_feat = Wbv @ bv_in[g] + bbv                      per graph (H, N)
    w_euc   = softmax((b_feat[g,:32].T @ f[:32]) / 8)   per node over N
    w_geo   = softmax((b_feat[g,32:].T @ f[32:]) / 8)
    out     = [bv_feat[g] @ w_euc, bv_feat[g] @ w_geo] @ Wo.T + bo   (M, H)

Sharding strategy: nodes are sorted by graph id, so split each graph's
node range into chunks of <=512 contiguous nodes.  Sum_g ceil(c_g/512)
is provably <= 16 for B=8 graphs and M=4096 nodes, so every one of the
8 cores gets exactly 2 chunks (dummy zero-filled chunks pad the tail).
Each chunk is single-graph, so on-device there is no gather at all:
the host stages, per chunk, the transposed node features f.T (128d x
512m), that chunk's graph boundary input b_pre_in[g] (128d x 512n) and
bv_in[g] (+ a constant ones channel used to fold the bbv bias and the
softmax-denominator column into matmuls).

Device layout is fully "transposed world": nodes m stay on the matmul
free dimension everywhere; softmax reductions over the N boundary
positions become matmul-column tricks (a ones column in the bv weight
produces the softmax denominator as row 64 of the apply matmul), and
the per-node normalization is applied via a rank-1 broadcast matmul.
Output is produced transposed, (H, m); the host transposes back.
"""

import sys

for _p in ("/opt/trn_rl_repo", "/root/.axon_site/_ro/trn_rl_repo"):
    if _p not in sys.path:
        sys.path.append(_p)

import numpy as np

import bass_rust

import concourse.bass as bass
import concourse.mybir as mybir
from concourse.bass_utils import run_bass_kernel_spmd
from concourse.tile import TileContext
from concourse.vector_clock import ScopedClock, VectorClock

F32 = mybir.dt.float32
F32R = mybir.dt.float32r


def _r(ap):
    """Reinterpret an fp32 AP as float32r for full-rate PE matmuls
    (single-pass, slightly reduced multiply precision)."""
    if ap.dtype == F32R:
        return ap
    return ap.bitcast(F32R)

# The walrus build in this environment rejects multiple semaphore waits
# on one instruction (matmuls fail even at 2), so carry every wait on its
# own nop ahead of the real instruction.
_MAX_WAITS = 1

# Problem shapes (hardcoded per the harness contract).
M, B, N, FD, BD, BVD, H = 4096, 8, 512, 128, 128, 6, 64
H2 = H // 2
N_CORES = 8
CH = 256                    # nodes per chunk
# worst-case chunk count is B + M/CH (sum of per-graph ceils), so this
# many chunks per core always suffices:
CPC = -(-(B + M // CH) // N_CORES)
NCH = N_CORES * CPC
# each score/exp round covers SK n-chunks so the PSUM score tile is
# (128, SK*CH) = 2 banks; 4/SK rounds cover all 512 boundary positions
SK = max(1, 1024 // CH)
NR = 4 // SK


class _ChunkedDrainTileContext(TileContext):
    """The walrus build in this environment rejects >2 semaphore waits on
    a single CTRL instruction, which breaks TileContext's final drain (it
    carries one wait per touched proc).  Split those waits across one SP
    nop per proc; SP executes serially, so a bare drain afterwards is
    equivalent."""

    _nop_uid = 0

    def _add_instruction(self, inst):
        si = inst.sync_info
        if (
            si is not None
            and si.on_wait
            and len(si.on_wait) > _MAX_WAITS
            and inst.engine != mybir.EngineType.Unassigned
        ):
            waits = list(si.on_wait)
            excess, keep = waits[:-_MAX_WAITS], waits[-_MAX_WAITS:]
            for i in range(0, len(excess), _MAX_WAITS):
                _ChunkedDrainTileContext._nop_uid += 1
                nop = mybir.InstNoOp(
                    name=f"splitw{_ChunkedDrainTileContext._nop_uid}", ins=[], outs=[]
                )
                nop.engine = inst.engine
                nop.sync_info = bass_rust.SyncInfo(
                    on_wait=excess[i : i + _MAX_WAITS], on_update=[]
                )
                super()._add_instruction(nop)
            inst.sync_info = bass_rust.SyncInfo(on_wait=keep, on_update=si.on_update)
        super()._add_instruction(inst)

    def _drain_and_barrier(self, tick_clock, wait_clock):
        nc = self.nc
        g = tick_clock.global_clock
        nprocs = len(g)
        for i in range(nprocs):
            if g[i] > 0:
                vc = VectorClock([0] * nprocs)
                vc.require_at_least(i, g[i])
                nop_inst = nc.sync.nop(nofuse=True, hint=f"drain_wait_p{i}")
                wait_clock.add_sem_waits(nop_inst.ins, ScopedClock({None: vc}))
        nc.sync.drain()
        nc.all_engine_barrier()
        assert self.sems is not None
        popped = nc._tile_sem_poison_stack.pop()
        assert popped is self._sem_poison
        nc.clear_and_free_semaphores(list(self.sems.allocated().values()))
        nc.all_engine_barrier()


def build_program(reps=1):
    """Build the per-core SPMD Bass program (identical on all 8 cores).

    reps>1 repeats the whole (idempotent) pipeline in-program; used only
    by the timing harness to amortize dispatch overhead."""
    nc = bass.Bass()

    # fb: per chunk [f.T | b_pre] fused into one (128, 1024) DMA
    d_fb = nc.declare_dram_parameter("fb", [CPC, FD, CH + N], F32R, isOutput=False)
    # bve: both chunks side by side, one DMA
    d_bv = nc.declare_dram_parameter("bve", [BVD + 1, CPC * N], F32R, isOutput=False)
    # wpk: every weight/bias/const packed into one (128, 388) DMA; column
    # layout: wft 0:64 | wbt 64:128 | wote 128:192 | wotg 192:256 |
    # ones 256:320 | wbve(rows 0:7) 320:385 | bf,bb,bo cols 385:388
    d_wpk = nc.declare_dram_parameter("wpk", [128, 388], F32R, isOutput=False)
    d_out = nc.declare_dram_parameter("outT", [CPC, H, CH], F32, isOutput=True)

    with _ChunkedDrainTileContext(nc) as tc, nc.allow_low_precision(
        reason="fp32r rounding of fp32 data"
    ):
        with (
            tc.tile_pool(name="const", bufs=1) as cp,
            tc.tile_pool(name="io", bufs=3) as iop,
            tc.tile_pool(name="wk", bufs=2) as wkp,
            tc.tile_pool(name="ex", bufs=4) as exp_pool,
            tc.tile_pool(name="ps_s", bufs=2, space="PSUM") as psp_s,
            tc.tile_pool(name="ps_m", bufs=1, space="PSUM") as psp_m,
            tc.tile_pool(name="ps_o", bufs=2, space="PSUM") as psp_o,
            tc.tile_pool(name="ps_r", bufs=1, space="PSUM") as psp_r,
        ):
            t_w = cp.tile([128, 388], F32R, tag="wpk")
            nc.sync.dma_start(t_w[:], d_wpk[:])
            t_wft = t_w[:, 0:64]
            t_wbt = t_w[:, 64:128]
            t_wote = t_w[0:H, 128:192]
            t_wotg = t_w[0:H, 192:256]
            t_ones_row = t_w[H : H + 1, 256:320]
            t_wbve = t_w[0 : BVD + 1, 320:385]
            t_bfc = t_w[0:H, 385:386].bitcast(F32)
            t_bbc = t_w[0:H, 386:387].bitcast(F32)
            t_boc = t_w[0:H, 387:388].bitcast(F32)
            t_bve = cp.tile([BVD + 1, CPC * N], F32R, tag="bve")
            nc.sync.dma_start(t_bve[:], d_bv[:])

            for rep in range(reps):
                # ---- phase 1: per-chunk loads + feature prep (f, b, bv) ----
                prep = []
                for c in range(CPC):
                    # separate tiles (and HWDGE queues) for f and b so the
                    # first feature matmul only waits on its own 256KB
                    t_ft = iop.tile([FD, CH], F32R, tag="f")
                    nc.sync.dma_start(t_ft[:], d_fb[c][:, 0:CH])
                    t_bt = iop.tile([BD, N], F32R, tag="b")
                    nc.scalar.dma_start(t_bt[:], d_fb[c][:, CH : CH + N])
                    t_f = t_ft[:]
                    t_b = t_bt[:]
                    t_bv = t_bve[:, N * c : N * (c + 1)]

                    # f_all (H, m) = Wf @ fT + bf
                    ps_f = psp_m.tile([H, CH], F32, tag="feat")
                    nc.tensor.matmul(
                        ps_f[:], _r(t_wft), _r(t_f), start=True, stop=True
                    )
                    t_fall = wkp.tile([H, CH], F32R, tag="fall")
                    nc.scalar.activation(
                        t_fall[:],
                        ps_f[:],
                        mybir.ActivationFunctionType.Identity,
                        bias=t_bfc,
                    )

                    # b_featT (H, n) = Wb @ b_pre + bb
                    ps_b = psp_m.tile([H, N], F32, tag="feat")
                    nc.tensor.matmul(
                        ps_b[:], _r(t_wbt), _r(t_b), start=True, stop=True
                    )
                    t_bf = wkp.tile([H, N], F32R, tag="bf")
                    nc.vector.tensor_scalar_add(t_bf[:], ps_b[:], t_bbc)

                    # bv_nh (n-chunk, H+1): col h = bv_feat[h,n]+bbv, col H = 1
                    # all four n-chunks fit in one PSUM bank -> one copy
                    ps_bv = psp_m.tile([128, 4 * (H + 1)], F32, tag="feat")
                    for j in range(4):
                        nc.tensor.matmul(
                            ps_bv[:, (H + 1) * j : (H + 1) * (j + 1)],
                            t_bv[:, 128 * j : 128 * (j + 1)].bitcast(F32),
                            t_wbve.bitcast(F32),
                            start=True,
                            stop=True,
                        )
                    tbv = wkp.tile([128, 4 * (H + 1)], F32R, tag="bvnh")
                    nc.vector.tensor_copy(tbv[:], ps_bv[:])
                    t_bvnh = [tbv[:, (H + 1) * j : (H + 1) * (j + 1)] for j in range(4)]
                    prep.append((t_fall, t_bf, t_bvnh))

                # ---- phase 2: scores -> exp -> apply, 2 rounds per half ----
                cats = []
                for c in range(CPC):
                    t_fall, t_bf, t_bvnh = prep[c]
                    t_cat = {}
                    for hx, h0 in (("e", 0), ("g", H2)):
                        ps_o = psp_o.tile([H + 1, CH], F32, tag="out")
                        for r in range(NR):
                            ps_s = psp_s.tile([128, SK * CH], F32, tag="s")
                            for jj in range(SK):
                                j = SK * r + jj
                                nc.tensor.matmul(
                                    ps_s[:, CH * jj : CH * (jj + 1)],
                                    _r(t_bf[h0 : h0 + H2, 128 * j : 128 * (j + 1)]),
                                    _r(t_fall[h0 : h0 + H2, :]),
                                    start=True,
                                    stop=True,
                                )
                            te = exp_pool.tile([128, SK * CH], F32R, tag="exp")
                            nc.scalar.activation(
                                te[:],
                                ps_s[:],
                                mybir.ActivationFunctionType.Exp,
                                scale=0.125,
                            )
                            for jj in range(SK):
                                j = SK * r + jj
                                nc.tensor.matmul(
                                    ps_o[:],
                                    _r(t_bvnh[j]),
                                    _r(te[:, CH * jj : CH * (jj + 1)]),
                                    start=(j == 0),
                                    stop=(j == 3),
                                )
                        # 1/colsum, broadcast to H partitions via rank-1 matmul
                        t_r = wkp.tile([H + 1, CH], F32R, tag=f"r{hx}")
                        nc.vector.reciprocal(t_r[H : H + 1, :], ps_o[H : H + 1, :])
                        ps_rb = psp_r.tile([H, CH], F32, tag="rb")
                        nc.tensor.matmul(
                            ps_rb[:],
                            _r(t_ones_row),
                            _r(t_r[H : H + 1, :]),
                            start=True,
                            stop=True,
                        )
                        t_rb = wkp.tile([H, CH], F32, tag=f"rb{hx}")
                        nc.scalar.copy(t_rb[:], ps_rb[:])
                        tcat = wkp.tile([H, CH], F32R, tag=f"cat{hx}")
                        nc.vector.tensor_mul(tcat[:], ps_o[0:H, :], t_rb[:])
                        t_cat[hx] = tcat
                    cats.append(t_cat)

                # ---- phase 3: final projection + store ----
                for c in range(CPC):
                    t_cat = cats[c]
                    ps_fin = psp_s.tile([H, CH], F32, tag="s")
                    nc.tensor.matmul(
                        ps_fin[:], _r(t_wote), _r(t_cat["e"][:]), start=True, stop=False
                    )
                    nc.tensor.matmul(
                        ps_fin[:], _r(t_wotg), _r(t_cat["g"][:]), start=False, stop=True
                    )
                    t_out = iop.tile([H, CH], F32, tag="out")
                    nc.vector.tensor_scalar_add(t_out[:], ps_fin[:], t_boc)
                    nc.sync.dma_start(d_out[c], t_out[:])

    return nc


def make_chunks(batch):
    """Split sorted per-node graph ids into <=NCH single-graph chunks of
    <=CH contiguous nodes.  Returns [(graph, node_offset, length)]."""
    batch = np.asarray(batch).astype(np.int64)
    bounds = np.searchsorted(batch, np.arange(B + 1))
    chunks = []
    for g in range(B):
        s, e = int(bounds[g]), int(bounds[g + 1])
        for off in range(s, e, CH):
            chunks.append((g, off, min(CH, e - off)))
    assert len(chunks) <= NCH, f"chunk overflow: {len(chunks)}"
    while len(chunks) < NCH:
        chunks.append((-1, 0, 0))
    return chunks


def stage_inputs(inputs, chunks):
    """Build the 8 per-core input maps from the full problem inputs."""
    f_pre_in = np.ascontiguousarray(np.asarray(inputs["f_pre_in"], dtype=np.float32))
    b_pre_in = np.ascontiguousarray(np.asarray(inputs["b_pre_in"], dtype=np.float32))
    bv_in = np.ascontiguousarray(np.asarray(inputs["bv_in"], dtype=np.float32))
    Wf = np.asarray(inputs["Wf"], dtype=np.float32)
    bf = np.asarray(inputs["bf"], dtype=np.float32)
    Wb = np.asarray(inputs["Wb"], dtype=np.float32)
    bb = np.asarray(inputs["bb"], dtype=np.float32)
    Wbv = np.asarray(inputs["Wbv"], dtype=np.float32)
    bbv = np.asarray(inputs["bbv"], dtype=np.float32)
    Wo = np.asarray(inputs["Wo"], dtype=np.float32)
    bo = np.asarray(inputs["bo"], dtype=np.float32)

    fb = np.zeros((NCH, FD, CH + N), np.float32)
    bve = np.zeros((NCH, BVD + 1, N), np.float32)
    bve[:, BVD, :] = 1.0
    for k, (g, off, L) in enumerate(chunks):
        if L > 0:
            fb[k, :, :L] = f_pre_in[off : off + L].T
            fb[k, :, CH : CH + N] = b_pre_in[g]
            bve[k, :BVD] = bv_in[g]

    wpk = np.zeros((128, 388), np.float32)
    wpk[:, 0:64] = Wf.T
    wpk[:, 64:128] = Wb.T
    wpk[0:H, 128:192] = Wo.T[0:H]
    wpk[0:H, 192:256] = Wo.T[H : 2 * H]
    wpk[:, 256:320] = 1.0
    wpk[0:BVD, 320:384] = Wbv.T
    wpk[BVD, 320:384] = bbv
    wpk[BVD, 384] = 1.0
    wpk[0:H, 385] = bf
    wpk[0:H, 386] = bb
    wpk[0:H, 387] = bo

    in_maps = []
    for c in range(N_CORES):
        m = {"wpk": wpk}
        m["fb"] = np.ascontiguousarray(fb[CPC * c : CPC * (c + 1)])
        m["bve"] = np.ascontiguousarray(
            np.concatenate(
                [bve[CPC * c + i] for i in range(CPC)], axis=1
            )
        )
        in_maps.append(m)
    return in_maps


def unstage_output(results, chunks):
    out = np.zeros((M, H), np.float32)
    for k, (g, off, L) in enumerate(chunks):
        if L > 0:
            core, cc = divmod(k, CPC)
            out[off : off + L] = results[core]["outT"][cc, :, :L].T
    return out


_NC_CACHE = []


def kernel(**inputs):
    assert np.asarray(inputs["f_pre_in"]).shape == (M, FD)
    chunks = make_chunks(inputs["f_pre_batch"])
    in_maps = stage_inputs(inputs, chunks)
    if not _NC_CACHE:
        _NC_CACHE.append(build_program())
    nc = _NC_CACHE[0]
    res = run_bass_kernel_spmd(nc, in_maps, core_ids=list(range(N_CORES)))
    return unstage_output(res.results, chunks)


if __name__ == "__main__":
    rng = np.random.default_rng(0)
    demo = {
        "f_pre_in": rng.standard_normal((M, FD), dtype=np.float32),
        "f_pre_batch": np.sort(rng.integers(0, B, size=M)),
        "b_pre_in": rng.standard_normal((B, BD, N), dtype=np.float32),
        "bv_in": rng.standard_normal((B, BVD, N), dtype=np.float32),
        "Wf": rng.standard_normal((H, FD), dtype=np.float32) * 0.05,
        "bf": rng.standard_normal(H, dtype=np.float32) * 0.05,
        "Wb": rng.standard_normal((H, BD), dtype=np.float32) * 0.05,
        "bb": rng.standard_normal(H, dtype=np.float32) * 0.05,
        "Wbv": rng.standard_normal((H, BVD), dtype=np.float32) * 0.05,
        "bbv": rng.standard_normal(H, dtype=np.float32) * 0.05,
        "Wo": rng.standard_normal((H, 2 * H), dtype=np.float32) * 0.05,
        "bo": rng.standard_normal(H, dtype=np.float32) * 0.05,
    }
    out = kernel(**demo)
    print("kernel output", out.shape, out.dtype, float(np.abs(out).mean()))


# revision 33
# speedup vs baseline: 1.3680x; 1.3680x over previous
"""Trainium2 Bass kernel for the ragged-sequence cross-attention module.

Math (reference):
    f       = Wf @ f_pre_in.T + bf                      (H, M)
    b_feat  = Wb @ b_pre_in[g] + bb                     per graph (H, N)
    bv_feat = Wbv @ bv_in[g] + bbv                      per graph (H, N)
    w_euc   = softmax((b_feat[g,:32].T @ f[:32]) / 8)   per node over N
    w_geo   = softmax((b_feat[g,32:].T @ f[32:]) / 8)
    out     = [bv_feat[g] @ w_euc, bv_feat[g] @ w_geo] @ Wo.T + bo   (M, H)

Sharding strategy: nodes are sorted by graph id, so split each graph's
node range into chunks of <=512 contiguous nodes.  Sum_g ceil(c_g/512)
is provably <= 16 for B=8 graphs and M=4096 nodes, so every one of the
8 cores gets exactly 2 chunks (dummy zero-filled chunks pad the tail).
Each chunk is single-graph, so on-device there is no gather at all:
the host stages, per chunk, the transposed node features f.T (128d x
512m), that chunk's graph boundary input b_pre_in[g] (128d x 512n) and
bv_in[g] (+ a constant ones channel used to fold the bbv bias and the
softmax-denominator column into matmuls).

Device layout is fully "transposed world": nodes m stay on the matmul
free dimension everywhere; softmax reductions over the N boundary
positions become matmul-column tricks (a ones column in the bv weight
produces the softmax denominator as row 64 of the apply matmul), and
the per-node normalization is applied via a rank-1 broadcast matmul.
Output is produced transposed, (H, m); the host transposes back.
"""

import sys

for _p in ("/opt/trn_rl_repo", "/root/.axon_site/_ro/trn_rl_repo"):
    if _p not in sys.path:
        sys.path.append(_p)

import numpy as np

import bass_rust

import concourse.bass as bass
import concourse.mybir as mybir
from concourse.bass_utils import run_bass_kernel_spmd
from concourse.tile import TileContext
from concourse.vector_clock import ScopedClock, VectorClock

F32 = mybir.dt.float32
F32R = mybir.dt.float32r


def _r(ap):
    """Reinterpret an fp32 AP as float32r for full-rate PE matmuls
    (single-pass, slightly reduced multiply precision)."""
    if ap.dtype == F32R:
        return ap
    return ap.bitcast(F32R)

# The walrus build in this environment rejects multiple semaphore waits
# on one instruction (matmuls fail even at 2), so carry every wait on its
# own nop ahead of the real instruction.
_MAX_WAITS = 1

# Problem shapes (hardcoded per the harness contract).
M, B, N, FD, BD, BVD, H = 4096, 8, 512, 128, 128, 6, 64
H2 = H // 2
N_CORES = 8
CH = 256                    # nodes per chunk
# worst-case chunk count is B + M/CH (sum of per-graph ceils), so this
# many chunks per core always suffices:
CPC = -(-(B + M // CH) // N_CORES)
NCH = N_CORES * CPC
# each score/exp round covers SK n-chunks so the PSUM score tile is
# (128, SK*CH) = 2 banks; 4/SK rounds cover all 512 boundary positions
SK = max(1, 1024 // CH)
NR = 4 // SK


class _ChunkedDrainTileContext(TileContext):
    """The walrus build in this environment rejects >2 semaphore waits on
    a single CTRL instruction, which breaks TileContext's final drain (it
    carries one wait per touched proc).  Split those waits across one SP
    nop per proc; SP executes serially, so a bare drain afterwards is
    equivalent."""

    _nop_uid = 0

    def _add_instruction(self, inst):
        si = inst.sync_info
        if (
            si is not None
            and si.on_wait
            and len(si.on_wait) > _MAX_WAITS
            and inst.engine != mybir.EngineType.Unassigned
        ):
            waits = list(si.on_wait)
            excess, keep = waits[:-_MAX_WAITS], waits[-_MAX_WAITS:]
            for i in range(0, len(excess), _MAX_WAITS):
                _ChunkedDrainTileContext._nop_uid += 1
                nop = mybir.InstNoOp(
                    name=f"splitw{_ChunkedDrainTileContext._nop_uid}", ins=[], outs=[]
                )
                nop.engine = inst.engine
                nop.sync_info = bass_rust.SyncInfo(
                    on_wait=excess[i : i + _MAX_WAITS], on_update=[]
                )
                super()._add_instruction(nop)
            inst.sync_info = bass_rust.SyncInfo(on_wait=keep, on_update=si.on_update)
        super()._add_instruction(inst)

    def _drain_and_barrier(self, tick_clock, wait_clock):
        nc = self.nc
        g = tick_clock.global_clock
        nprocs = len(g)
        for i in range(nprocs):
            if g[i] > 0:
                vc = VectorClock([0] * nprocs)
                vc.require_at_least(i, g[i])
                nop_inst = nc.sync.nop(nofuse=True, hint=f"drain_wait_p{i}")
                wait_clock.add_sem_waits(nop_inst.ins, ScopedClock({None: vc}))
        nc.sync.drain()
        nc.all_engine_barrier()
        assert self.sems is not None
        popped = nc._tile_sem_poison_stack.pop()
        assert popped is self._sem_poison
        nc.clear_and_free_semaphores(list(self.sems.allocated().values()))
        nc.all_engine_barrier()


def build_program(reps=1):
    """Build the per-core SPMD Bass program (identical on all 8 cores).

    reps>1 repeats the whole (idempotent) pipeline in-program; used only
    by the timing harness to amortize dispatch overhead."""
    nc = bass.Bass()

    # fb: per chunk [f.T | b_pre] fused into one (128, 1024) DMA
    d_fb = nc.declare_dram_parameter("fb", [CPC, FD, CH + N], F32R, isOutput=False)
    # bve: both chunks side by side, one DMA
    d_bv = nc.declare_dram_parameter("bve", [BVD + 1, CPC * N], F32R, isOutput=False)
    # wpk: every weight/bias/const packed into one (128, 388) DMA; column
    # layout: wft 0:64 | wbt 64:128 | wote 128:192 | wotg 192:256 |
    # ones 256:320 | wbve(rows 0:7) 320:385 | bf,bb,bo cols 385:388
    d_wpk = nc.declare_dram_parameter("wpk", [128, 388], F32R, isOutput=False)
    d_out = nc.declare_dram_parameter("outT", [CPC, H, CH], F32, isOutput=True)

    with _ChunkedDrainTileContext(nc) as tc, nc.allow_low_precision(
        reason="fp32r rounding of fp32 data"
    ):
        with (
            tc.tile_pool(name="const", bufs=1) as cp,
            tc.tile_pool(name="io", bufs=3) as iop,
            tc.tile_pool(name="wk", bufs=3) as wkp,
            tc.tile_pool(name="ex", bufs=6) as exp_pool,
            tc.tile_pool(name="ps_s", bufs=2, space="PSUM") as psp_s,
            tc.tile_pool(name="ps_m", bufs=1, space="PSUM") as psp_m,
            tc.tile_pool(name="ps_o", bufs=2, space="PSUM") as psp_o,
            tc.tile_pool(name="ps_r", bufs=1, space="PSUM") as psp_r,
        ):
            t_w = cp.tile([128, 388], F32R, tag="wpk")
            nc.sync.dma_start(t_w[:], d_wpk[:])
            t_wft = t_w[:, 0:64]
            t_wbt = t_w[:, 64:128]
            t_wote = t_w[0:H, 128:192]
            t_wotg = t_w[0:H, 192:256]
            t_ones_row = t_w[H : H + 1, 256:320]
            t_wbve = t_w[0 : BVD + 1, 320:385]
            t_bfc = t_w[0:H, 385:386].bitcast(F32)
            t_bbc = t_w[0:H, 386:387].bitcast(F32)
            t_boc = t_w[0:H, 387:388].bitcast(F32)
            t_bve = cp.tile([BVD + 1, CPC * N], F32R, tag="bve")
            nc.sync.dma_start(t_bve[:], d_bv[:])

            for rep in range(reps):
                # ---- phase 1: per-chunk loads + feature prep (f, b, bv) ----
                prep = []
                for c in range(CPC):
                    # separate tiles (and HWDGE queues) for f and b so the
                    # first feature matmul only waits on its own 256KB
                    t_ft = iop.tile([FD, CH], F32R, tag="f")
                    nc.sync.dma_start(t_ft[:], d_fb[c][:, 0:CH])
                    t_bt = iop.tile([BD, N], F32R, tag="b")
                    nc.scalar.dma_start(t_bt[:], d_fb[c][:, CH : CH + N])
                    t_f = t_ft[:]
                    t_b = t_bt[:]
                    t_bv = t_bve[:, N * c : N * (c + 1)]

                    # f_all (H, m) = Wf @ fT + bf
                    ps_f = psp_m.tile([H, CH], F32, tag="feat")
                    nc.tensor.matmul(
                        ps_f[:], _r(t_wft), _r(t_f), start=True, stop=True
                    )
                    t_fall = wkp.tile([H, CH], F32R, tag="fall")
                    nc.scalar.activation(
                        t_fall[:],
                        ps_f[:],
                        mybir.ActivationFunctionType.Identity,
                        bias=t_bfc,
                    )

                    # b_featT (H, n) = Wb @ b_pre + bb
                    ps_b = psp_m.tile([H, N], F32, tag="feat")
                    nc.tensor.matmul(
                        ps_b[:], _r(t_wbt), _r(t_b), start=True, stop=True
                    )
                    t_bf = wkp.tile([H, N], F32R, tag="bf")
                    nc.vector.tensor_scalar_add(t_bf[:], ps_b[:], t_bbc)

                    # bv_nh (n-chunk, H+1): col h = bv_feat[h,n]+bbv, col H = 1
                    # all four n-chunks fit in one PSUM bank -> one copy
                    ps_bv = psp_m.tile([128, 4 * (H + 1)], F32, tag="feat")
                    for j in range(4):
                        nc.tensor.matmul(
                            ps_bv[:, (H + 1) * j : (H + 1) * (j + 1)],
                            t_bv[:, 128 * j : 128 * (j + 1)].bitcast(F32),
                            t_wbve.bitcast(F32),
                            start=True,
                            stop=True,
                        )
                    tbv = wkp.tile([128, 4 * (H + 1)], F32R, tag="bvnh")
                    nc.vector.tensor_copy(tbv[:], ps_bv[:])
                    t_bvnh = [tbv[:, (H + 1) * j : (H + 1) * (j + 1)] for j in range(4)]
                    prep.append((t_fall, t_bf, t_bvnh))

                # ---- phase 2: scores -> exp -> apply, 2 rounds per half ----
                cats = []
                for c in range(CPC):
                    t_fall, t_bf, t_bvnh = prep[c]
                    t_cat = {}
                    for hx, h0 in (("e", 0), ("g", H2)):
                        ps_o = psp_o.tile([H + 1, CH], F32, tag="out")
                        for r in range(NR):
                            ps_s = psp_s.tile([128, SK * CH], F32, tag="s")
                            for jj in range(SK):
                                j = SK * r + jj
                                nc.tensor.matmul(
                                    ps_s[:, CH * jj : CH * (jj + 1)],
                                    _r(t_bf[h0 : h0 + H2, 128 * j : 128 * (j + 1)]),
                                    _r(t_fall[h0 : h0 + H2, :]),
                                    start=True,
                                    stop=True,
                                )
                            te = exp_pool.tile([128, SK * CH], F32R, tag="exp")
                            nc.scalar.activation(
                                te[:],
                                ps_s[:],
                                mybir.ActivationFunctionType.Exp,
                                scale=0.125,
                            )
                            for jj in range(SK):
                                j = SK * r + jj
                                nc.tensor.matmul(
                                    ps_o[:],
                                    _r(t_bvnh[j]),
                                    _r(te[:, CH * jj : CH * (jj + 1)]),
                                    start=(j == 0),
                                    stop=(j == 3),
                                )
                        # 1/colsum, broadcast to H partitions via rank-1 matmul
                        t_r = wkp.tile([H + 1, CH], F32R, tag=f"r{hx}")
                        nc.vector.reciprocal(t_r[H : H + 1, :], ps_o[H : H + 1, :])
                        ps_rb = psp_r.tile([H, CH], F32, tag="rb")
                        nc.tensor.matmul(
                            ps_rb[:],
                            _r(t_ones_row),
                            _r(t_r[H : H + 1, :]),
                            start=True,
                            stop=True,
                        )
                        t_rb = wkp.tile([H, CH], F32, tag=f"rb{hx}")
                        # balance the broadcast copies across DVE and ACT
                        if hx == "e":
                            nc.vector.tensor_copy(t_rb[:], ps_rb[:])
                        else:
                            nc.scalar.copy(t_rb[:], ps_rb[:])
                        tcat = wkp.tile([H, CH], F32R, tag=f"cat{hx}")
                        nc.vector.tensor_mul(tcat[:], ps_o[0:H, :], t_rb[:])
                        t_cat[hx] = tcat
                    cats.append(t_cat)

                # ---- phase 3: final projection + store ----
                for c in range(CPC):
                    t_cat = cats[c]
                    ps_fin = psp_s.tile([H, CH], F32, tag="s")
                    nc.tensor.matmul(
                        ps_fin[:], _r(t_wote), _r(t_cat["e"][:]), start=True, stop=False
                    )
                    nc.tensor.matmul(
                        ps_fin[:], _r(t_wotg), _r(t_cat["g"][:]), start=False, stop=True
                    )
                    t_out = iop.tile([H, CH], F32, tag="out")
                    nc.vector.tensor_scalar_add(t_out[:], ps_fin[:], t_boc)
                    nc.sync.dma_start(d_out[c], t_out[:])

    return nc


def make_chunks(batch):
    """Split sorted per-node graph ids into <=NCH single-graph chunks of
    <=CH contiguous nodes.  Returns [(graph, node_offset, length)]."""
    batch = np.asarray(batch).astype(np.int64)
    bounds = np.searchsorted(batch, np.arange(B + 1))
    chunks = []
    for g in range(B):
        s, e = int(bounds[g]), int(bounds[g + 1])
        for off in range(s, e, CH):
            chunks.append((g, off, min(CH, e - off)))
    assert len(chunks) <= NCH, f"chunk overflow: {len(chunks)}"
    while len(chunks) < NCH:
        chunks.append((-1, 0, 0))
    return chunks


def stage_inputs(inputs, chunks):
    """Build the 8 per-core input maps from the full problem inputs."""
    f_pre_in = np.ascontiguousarray(np.asarray(inputs["f_pre_in"], dtype=np.float32))
    b_pre_in = np.ascontiguousarray(np.asarray(inputs["b_pre_in"], dtype=np.float32))
    bv_in = np.ascontiguousarray(np.asarray(inputs["bv_in"], dtype=np.float32))
    Wf = np.asarray(inputs["Wf"], dtype=np.float32)
    bf = np.asarray(inputs["bf"], dtype=np.float32)
    Wb = np.asarray(inputs["Wb"], dtype=np.float32)
    bb = np.asarray(inputs["bb"], dtype=np.float32)
    Wbv = np.asarray(inputs["Wbv"], dtype=np.float32)
    bbv = np.asarray(inputs["bbv"], dtype=np.float32)
    Wo = np.asarray(inputs["Wo"], dtype=np.float32)
    bo = np.asarray(inputs["bo"], dtype=np.float32)

    fb = np.zeros((NCH, FD, CH + N), np.float32)
    bve = np.zeros((NCH, BVD + 1, N), np.float32)
    bve[:, BVD, :] = 1.0
    for k, (g, off, L) in enumerate(chunks):
        if L > 0:
            fb[k, :, :L] = f_pre_in[off : off + L].T
            fb[k, :, CH : CH + N] = b_pre_in[g]
            bve[k, :BVD] = bv_in[g]

    wpk = np.zeros((128, 388), np.float32)
    wpk[:, 0:64] = Wf.T
    wpk[:, 64:128] = Wb.T
    wpk[0:H, 128:192] = Wo.T[0:H]
    wpk[0:H, 192:256] = Wo.T[H : 2 * H]
    wpk[:, 256:320] = 1.0
    wpk[0:BVD, 320:384] = Wbv.T
    wpk[BVD, 320:384] = bbv
    wpk[BVD, 384] = 1.0
    wpk[0:H, 385] = bf
    wpk[0:H, 386] = bb
    wpk[0:H, 387] = bo

    in_maps = []
    for c in range(N_CORES):
        m = {"wpk": wpk}
        m["fb"] = np.ascontiguousarray(fb[CPC * c : CPC * (c + 1)])
        m["bve"] = np.ascontiguousarray(
            np.concatenate(
                [bve[CPC * c + i] for i in range(CPC)], axis=1
            )
        )
        in_maps.append(m)
    return in_maps


def unstage_output(results, chunks):
    out = np.zeros((M, H), np.float32)
    for k, (g, off, L) in enumerate(chunks):
        if L > 0:
            core, cc = divmod(k, CPC)
            out[off : off + L] = results[core]["outT"][cc, :, :L].T
    return out


_NC_CACHE = []


def kernel(**inputs):
    assert np.asarray(inputs["f_pre_in"]).shape == (M, FD)
    chunks = make_chunks(inputs["f_pre_batch"])
    in_maps = stage_inputs(inputs, chunks)
    if not _NC_CACHE:
        _NC_CACHE.append(build_program())
    nc = _NC_CACHE[0]
    res = run_bass_kernel_spmd(nc, in_maps, core_ids=list(range(N_CORES)))
    return unstage_output(res.results, chunks)


if __name__ == "__main__":
    rng = np.random.default_rng(0)
    demo = {
        "f_pre_in": rng.standard_normal((M, FD), dtype=np.float32),
        "f_pre_batch": np.sort(rng.integers(0, B, size=M)),
        "b_pre_in": rng.standard_normal((B, BD, N), dtype=np.float32),
        "bv_in": rng.standard_normal((B, BVD, N), dtype=np.float32),
        "Wf": rng.standard_normal((H, FD), dtype=np.float32) * 0.05,
        "bf": rng.standard_normal(H, dtype=np.float32) * 0.05,
        "Wb": rng.standard_normal((H, BD), dtype=np.float32) * 0.05,
        "bb": rng.standard_normal(H, dtype=np.float32) * 0.05,
        "Wbv": rng.standard_normal((H, BVD), dtype=np.float32) * 0.05,
        "bbv": rng.standard_normal(H, dtype=np.float32) * 0.05,
        "Wo": rng.standard_normal((H, 2 * H), dtype=np.float32) * 0.05,
        "bo": rng.standard_normal(H, dtype=np.float32) * 0.05,
    }
    out = kernel(**demo)
    print("kernel output", out.shape, out.dtype, float(np.abs(out).mean()))
